# revision 1
# baseline (speedup 1.0000x reference)
"""Trainium2 Bass kernel for nn_BottleneckBit (ResNet bottleneck with ternary-
quantized convs + BN + SiLU + residual).

Strategy:
- Data-parallel over batch: 64 images -> 8 cores x 8 images.
- All convs lowered to TensorEngine matmuls with channels on partitions:
    conv1 (1x1, 1024->256):  8 K-tiles of 128 bf16, k-OUTER so the PE starts
                             streaming as soon as the first x chunk lands
    conv2 (3x3, 256->256):   DoubleRow fp8: both 128-channel halves contract in
                             one pass (2 fp8 weights/cell). 9 shifted-tap DR
                             matmuls per output tile, reading a zero-padded
                             16x16-per-image fp8 buffer with image-pair pixels
                             interleaved (n = r*28 + c*2 + i) so the shifted
                             window is a 4D AP [p, half, r, colpair].
    conv3 (1x1, 256->1024):  2 K-tiles bf16, a3 folded into the weights and c3
                             folded into the residual x' = x + c3 on the host
                             (conv1 corrected exactly via c1' = c1 - a1*(t1@c3),
                             CIN == COUT). Two epilogue flavors, placed so
                             every engine stays busy: where conv2 keeps the PE
                             hot, the residual rides one pair-batched DVE
                             tensor_add psum->stage followed by a quad
                             pure-silu ACT; in the tail the residual is an
                             identity matmul on the (otherwise idle) PE with a
                             pair ACT straight from PSUM.
- conv1 runs channels 512..1023 as two DoubleRow e4m3 k-pairs (x quantization
  error budgeted via simulation); its epilogue is a per-j DVE affine into an
  interleaved f32 stage + one pair-batched fp8 ACT per bp.
- conv3/l3 units are woven between conv2 tap blocks so the l3 epilogue engines
  (DVE/ACT) overlap conv2's PE time and PSUM slots recycle without stalling
  the in-order PE queue. Small dummy matmuls bridge every potential PE idle
  window so the HAM clock gate never drops the PE back to 1.2GHz.
- Ternary weight trick: wq = clip(round(w/s),-1,1)*s. The {-1,0,1} ternary part
  is exact in bf16/fp8; per-out-channel scale s and BN fold into (a, c).
- h1 is stored e4m3 with the image-pair/channel-half pixels interleaved
  (offset r*64 + c*4 + i*2 + e) so the DoubleRow moving operand reads its fp8
  pair in one 16-bit access — without this DR matmuls run at half speed.
- DMA order: w1 k0-slice + x stream first (L1 critical path), w2/w3 behind x,
  the residual-only half of bf16 x last.
"""
import numpy as np
import ml_dtypes

import concourse.bass as bass
import concourse.mybir as mybir
from concourse import bacc
from concourse.tile import TileContext
from concourse.bass_utils import run_bass_kernel_spmd
from concourse.masks import make_identity


BN_EPS = 1e-5
Q_EPS = 1e-8

# Problem shape (hardcoded per contract)
B, CIN, H, W = 64, 1024, 14, 14
WIDTH, COUT = 256, 1024
N_CORES = 8
BC = B // N_CORES          # images per core = 8
PIX = H * W                # 196
P = 128
CIN_T = CIN // P           # 8
W_T = WIDTH // P           # 2
COUT_T = COUT // P         # 8
IPG = 2                    # images per matmul group (pixel-interleaved)
BP = BC // IPG             # 4 image-pair groups per core
NN = IPG * PIX             # 392 columns per matmul
HP, WP = H + 2, W + 2      # 16x16 padded image for the 3x3 conv
RW = IPG * WP              # 32 interleaved row width
NDUMMY = 90                # 64-col pre-warm matmuls (cover the DMA lead-in)

_F32 = mybir.dt.float32
_BF16 = mybir.dt.bfloat16
_FP8 = mybir.dt.float8e4
_AF = mybir.ActivationFunctionType
_ALU = mybir.AluOpType
_DR = mybir.MatmulPerfMode.DoubleRow


def build(act_func=None):
    """Build the per-core Bass program (SPMD: same program on all 8 cores)."""
    if act_func is None:
        act_func = _AF.Silu
    nc = bacc.Bacc()

    xd = nc.declare_dram_parameter("x", [P, CIN_T * BC * PIX], _BF16, isOutput=False)
    # e4m3 copy of x channels 512..1023 for conv1's two DoubleRow k-pairs,
    # pair-interleaved: xq[p, kp*2*BC*PIX + n*2 + e] = x[(4+2kp+e)*128+p, n]
    xqd = nc.declare_dram_parameter("xq", [P, 2 * BC * PIX * 2], _FP8, isOutput=False)
    w1d = nc.declare_dram_parameter("w1", [P, CIN_T * WIDTH], _FP8, isOutput=False)
    w2d = nc.declare_dram_parameter("w2", [P, W_T * 9 * WIDTH], _FP8, isOutput=False)
    w3d = nc.declare_dram_parameter("w3", [P, W_T * COUT], _BF16, isOutput=False)
    ccd = nc.declare_dram_parameter("cc", [P, 8 + COUT_T], _F32, isOutput=False)
    # output stays in the partition-folded layout [p, j*BC*PIX + n]; host unfolds
    outd = nc.declare_dram_parameter("out", [P, COUT_T * BC * PIX], _BF16, isOutput=True)

    with TileContext(nc) as tc:
        with tc.tile_pool(name="weights", bufs=1) as wpool, \
             tc.tile_pool(name="acts", bufs=1) as apool, \
             tc.tile_pool(name="outs", bufs=4) as opool, \
             tc.tile_pool(name="stage", bufs=3) as stpool, \
             tc.tile_pool(name="psum", bufs=4, space="PSUM") as pspool:

            # ---- DMA order = critical path order. sync queue: w1 k0-slice,
            # then the full x stream, then w2 (j0, j1 blocks), then w3.
            # gpsimd queue: consts + the rest of w1 (needed from k=1) ----
            w1t = wpool.tile([P, CIN_T * WIDTH], _FP8, name="w1t")
            nc.sync.dma_start(out=w1t[:, 0:WIDTH], in_=w1d[:, 0:WIDTH])
            xt = apool.tile([P, CIN_T * BC * PIX], _BF16, name="xt")
            nc.sync.dma_start(out=xt[:, 0:NN], in_=xd[:, 0:NN])
            nc.sync.dma_start(out=xt[:, NN:BC * PIX], in_=xd[:, NN:BC * PIX])
            nc.sync.dma_start(out=xt[:, BC * PIX:2 * BC * PIX],
                              in_=xd[:, BC * PIX:2 * BC * PIX])
            nc.sync.dma_start(out=xt[:, 2 * BC * PIX:4 * BC * PIX],
                              in_=xd[:, 2 * BC * PIX:4 * BC * PIX])
            cct = wpool.tile([P, 8 + COUT_T], _F32, name="cct")
            nc.gpsimd.dma_start(out=cct[:, :], in_=ccd[:, :])
            nc.gpsimd.dma_start(out=w1t[:, WIDTH:], in_=w1d[:, WIDTH:])
            xqt = apool.tile([P, 2 * BC * PIX * 2], _FP8, name="xqt")
            nc.sync.dma_start(out=xqt[:, :], in_=xqd[:, :])
            w2t = wpool.tile([P, W_T * 9 * WIDTH], _FP8, name="w2t")
            half2 = 9 * W_T * P
            nc.sync.dma_start(out=w2t[:, 0:half2], in_=w2d[:, 0:half2])
            nc.sync.dma_start(out=w2t[:, half2:], in_=w2d[:, half2:])
            w3t = wpool.tile([P, W_T * COUT], _BF16, name="w3t")
            nc.sync.dma_start(out=w3t[:, :], in_=w3d[:, :])
            # bf16 x channels 512.. are only the conv3 residual; they arrive
            # behind the compute-critical tensors
            nc.sync.dma_start(out=xt[:, 4 * BC * PIX:],
                              in_=xd[:, 4 * BC * PIX:])

            def xs(t, bp):          # x slice [128, NN] for (channel tile, bpair)
                return xt[:, t * BC * PIX + bp * NN: t * BC * PIX + (bp + 1) * NN]

            # ---- PE clock pre-warm: HAM needs ~3.4us of sustained PE activity
            # to lift the 1.2->2.4GHz clock gate, and the first x chunk only
            # lands ~4us after the queues start. Dummy matmuls bridge the whole
            # DMA lead-in so L1 starts warm ----
            wsrc = apool.tile([P, 512], _BF16, name="wsrc")
            nc.vector.memset(wsrc[:, :], 0.0)
            identt = wpool.tile([P, P], _BF16, name="identt")
            make_identity(nc, identt[:, :])
            wps = pspool.tile([P, 1024], _F32, name="wps", tag="ps")
            for _ in range(NDUMMY):
                nc.tensor.matmul(wps[0:64, 0:64], wsrc[:, 0:64],
                                 wsrc[:, 0:64], start=True, stop=True)

            # ---- padded h1 buffers, fp8. Layout per bp group:
            # offset = r*64 + c*4 + i*2 + e  (e = channel half INNERMOST so
            # the DoubleRow moving operand reads its fp8 pair in one 16-bit
            # access; without this the DR matmul runs at half speed) ----
            h1p = []
            for bp in range(BP):
                t = apool.tile([P, HP * WP * IPG * W_T], _FP8, name=f"h1p{bp}")
                nc.vector.memset(t[:, :], 0.0)
                h1p.append(t)
            h2 = [apool.tile([P, W_T * NN], _BF16, name=f"h2_{bp}")
                  for bp in range(BP)]

            # ---- layer 1: 1x1 conv 1024->256. Channels 0..511 in bf16
            # (k-OUTER over the arriving x chunks); channels 512..1023 as two
            # DoubleRow e4m3 k-pairs, per-bp so each group finishes as soon as
            # its data is in. Epilogue: per-j DVE affine (a1*ps+c1) writes an
            # interleaved f32 stage; one pair-batched pure-silu ACT per bp
            # then emits the fp8 padded h1 (keeps the scalar engine off the
            # L1->L2 critical path) ----
            ps1 = [pspool.tile([P, 1024], _F32, name=f"ps1_{bp}", tag="ps")
                   for bp in range(BP)]
            w1v = w1t.rearrange("p (k m) -> p k m", k=CIN_T)
            xqv = xqt.rearrange("p (kp n e) -> p kp e n", kp=2, e=2)
            l1st = {}

            def l1_mm(k, bp, j):
                nc.tensor.matmul(
                    ps1[bp][:, j * 512: j * 512 + NN],
                    w1t[:, k * WIDTH + j * P: k * WIDTH + (j + 1) * P],
                    xs(k, bp),
                    start=(k == 0), stop=False)

            def l1_mm_dr(kp, bp, j):
                nc.tensor.matmul(
                    ps1[bp][:, j * 512: j * 512 + NN],
                    w1v[:, 4 + 2 * kp:6 + 2 * kp, j * P:(j + 1) * P],
                    xqv[:, kp, :, bp * NN:(bp + 1) * NN],
                    start=False, stop=(kp == 1), perf_mode=_DR)

            def l1_affine(bp, j):
                dst = l1st[bp].rearrange("p (n e) -> p n e", e=2)[:, :, j]
                nc.vector.tensor_scalar(
                    out=dst, in0=ps1[bp][:, j * 512: j * 512 + NN],
                    scalar1=cct[:, 0 + j:1 + j], scalar2=cct[:, 2 + j:3 + j],
                    op0=_ALU.mult, op1=_ALU.add)

            def l1_act(bp):
                src = l1st[bp].rearrange("p (r c ie) -> p r c ie", r=H, c=W)
                dst = h1p[bp].rearrange(
                    "p (r c ie) -> p r c ie", r=HP, c=WP)[:, 1:1 + H, 1:1 + W, :]
                nc.scalar.activation(dst, src, act_func)

            # bf16 k0/k1 stream first, the DoubleRow k-pairs in the middle
            # (their fp8 chunk lands between the bf16 ones), then k2; the last
            # bf16 k-tile goes bp-by-bp with the DVE affine + fp8 pair-ACT
            # right behind
            for k in range(4):
                for bp in range(BP):
                    for j in range(W_T):
                        l1_mm(k, bp, j)
            for bp in range(BP):
                l1st[bp] = stpool.tile([P, 2 * NN], _F32, name=f"l1st{bp}",
                                       tag="l1st")
                for j in range(W_T):
                    for kp in range(2):
                        l1_mm_dr(kp, bp, j)
                    l1_affine(bp, j)
                l1_act(bp)

            # ---- layer 2: 3x3 conv 256->256 via 9 shifted-tap DoubleRow fp8
            # matmuls. bp pairs share weight loads; emission is split into tap
            # blocks so l3 units can be woven between them ----
            w2v = w2t.rearrange("p (j t e m) -> p j t e m", j=W_T, t=9, e=W_T)
            ps2 = {}

            def l2_alloc(bpa, bpb):
                for bp in (bpa, bpb):
                    ps2[bp] = pspool.tile([P, 1024], _F32, name=f"ps2_{bp}",
                                          tag="ps")

            def l2_tap_mm(bp, j, tap):
                dy, dx = divmod(tap, 3)
                rhs = h1p[bp].rearrange(
                    "p (r ci e) -> p e r ci", r=HP, e=W_T
                )[:, :, dy:dy + H, IPG * dx:IPG * dx + IPG * W]
                nc.tensor.matmul(
                    ps2[bp][:, j * 512: j * 512 + NN],
                    w2v[:, j, tap], rhs,
                    start=(tap == 0), stop=(tap == 8), perf_mode=_DR)

            def l2_taps(bpa, bpb, j, t0, t1):
                for tap in range(t0, t1):
                    for bp in (bpa, bpb):
                        l2_tap_mm(bp, j, tap)

            def l2_acts(bpa, bpb):
                # per-j affine (a2*ps + c2) psum->stage on the DVE (emitted
                # after ALL taps so no write-after-read hazard ever touches
                # the in-flight j1 matmuls; also frees the psum slot early),
                # then one pair-batched pure-silu ACT per bp from the stage
                for bp in (bpa, bpb):
                    l2st = stpool.tile([P, 2 * NN], _F32, name="l2st",
                                       tag="l2st")
                    for j in range(W_T):
                        nc.vector.tensor_scalar(
                            out=l2st[:, j * NN:(j + 1) * NN],
                            in0=ps2[bp][:, j * 512: j * 512 + NN],
                            scalar1=cct[:, 4 + j:5 + j],
                            scalar2=cct[:, 6 + j:7 + j],
                            op0=_ALU.mult, op1=_ALU.add)
                    nc.scalar.activation(h2[bp][:, :], l2st[:, :], act_func)

            # ---- layer 3: 1x1 conv 256->1024 bf16 (a3 folded into weights).
            # One "unit" = a j-pair: 4 conv matmuls + epilogue + pair ACT.
            # mode 'dve': residual + c3 in one DVE scalar_tensor_tensor
            # mode 'pe':  residual via identity matmul, c3 via DVE tensor_scalar
            # ot batches 4 j-tiles per out DMA ----
            # The residual x already carries c3 (folded on the host), so the
            # epilogue per j-pair is either ONE pair-batched DVE add
            # psum->stage + a quad pure-silu ACT (mode 'stage' — used where
            # conv2 keeps the PE busy), or an identity matmul on the PE with a
            # pair ACT straight from PSUM (mode 'pe' — used in the tail where
            # the PE would idle; zero DVE work). keep_warm prepends tiny dummy
            # matmuls so the HAM clock gate stays at 2.4GHz across epilogue
            # waits. dma='pair' flushes half-size DMAs to shorten the tail.
            def l3_units(bp, modes='ssss', dma='quad', keep_warm=0,
                         dma_eng=None):
                state = {}

                def mk(j0, mode):
                    def emit():
                        if mode == 'stage' and j0 % 4 == 0:
                            state['st'] = stpool.tile([P, 4 * NN], _F32,
                                                      name="st", tag="st")
                        if j0 % 4 == 0:
                            state['ot'] = opool.tile([P, 4 * NN], _BF16,
                                                     name="ot", tag="ot")
                        ot = state['ot']
                        ps3 = pspool.tile([P, 1024], _F32, name="ps3", tag="ps")
                        # keep-warm dummies land in this unit's own psum and
                        # are overwritten by the start=True conv matmul below
                        for _ in range(keep_warm):
                            nc.tensor.matmul(ps3[:, 0:P], wsrc[:, 0:P],
                                             wsrc[:, 0:P], start=True,
                                             stop=True)
                        for dj in range(2):
                            j = j0 + dj
                            sl = ps3[:, dj * 512: dj * 512 + NN]
                            for k in range(W_T):
                                nc.tensor.matmul(
                                    sl,
                                    w3t[:, k * COUT + j * P:
                                        k * COUT + (j + 1) * P],
                                    h2[bp][:, k * NN:(k + 1) * NN],
                                    start=(k == 0),
                                    stop=(mode == 'stage' and k == W_T - 1))
                            if mode == 'pe':
                                nc.tensor.matmul(sl, identt[:, :], xs(j, bp),
                                                 start=False, stop=True)
                        pspair = ps3.rearrange("p (g n) -> p g n", g=2)[
                            :, :, 0:NN]
                        otpair = ot.rearrange("p (g n) -> p g n", g=4)[
                            :, (j0 % 4):(j0 % 4) + 2, :]
                        if mode == 'stage':
                            xpair = xt.rearrange(
                                "p (t n) -> p t n", t=CIN_T)[
                                :, j0:j0 + 2, bp * NN:(bp + 1) * NN]
                            stpair = state['st'][
                                :, (j0 % 4) * NN:(j0 % 4 + 2) * NN
                            ].rearrange("p (g n) -> p g n", g=2)
                            nc.vector.tensor_add(out=stpair, in0=pspair,
                                                 in1=xpair)
                            if j0 % 4 == 2:
                                nc.scalar.activation(ot[:, :],
                                                     state['st'][:, :],
                                                     act_func)
                        else:
                            nc.scalar.activation(otpair, pspair, act_func)
                        if dma == 'pair' or j0 % 4 == 2:
                            jlo = j0 if dma == 'pair' else j0 - 2
                            nj = 2 if dma == 'pair' else 4
                            dmadst = outd.rearrange(
                                "p (j n) -> p j n", j=COUT_T)[
                                :, jlo:jlo + nj, bp * NN:(bp + 1) * NN]
                            src = ot.rearrange("p (g n) -> p g n", g=4)[
                                :, (jlo % 4):(jlo % 4) + nj, :]
                            (dma_eng or nc.sync).dma_start(out=dmadst, in_=src)
                    return emit
                return [mk(j0, {'s': 'stage', 'p': 'pe'}[m])
                        for j0, m in zip((0, 2, 4, 6), modes)]

            # ---- emission schedule: pair01 solid; l3(0)/l3(1) units woven
            # into pair23's tap blocks; l3(2)/l3(3) as the PE-resid tail ----
            l2_alloc(0, 1)
            # a few keep-warm fillers absorb the L1-epilogue chain latency
            for _ in range(12):
                nc.tensor.matmul(ps2[0][0:64, 0:64], wsrc[:, 0:64],
                                 wsrc[:, 0:64], start=True, stop=True)
            for j in range(W_T):
                l2_taps(0, 1, j, 0, 9)
            l2_acts(0, 1)
            l2_alloc(2, 3)
            u0 = l3_units(0, modes='sspp')
            u1 = l3_units(1, modes='sspp')
            l2_taps(2, 3, 0, 0, 5)
            u0[0]()
            u1[0]()
            l2_taps(2, 3, 0, 5, 9)
            u0[1]()
            u1[1]()
            l2_taps(2, 3, 1, 0, 5)
            u0[2]()
            u1[2]()
            l2_taps(2, 3, 1, 5, 9)
            l2_acts(2, 3)
            u0[3]()
            u1[3]()
            for u in l3_units(2, modes='sspp', keep_warm=1):
                u()
            for u in l3_units(3, modes='pppp', dma='pair'):
                u()

    nc.finalize()
    return nc


def _prep_host(x, w1, b1, g1, be1, m1, v1,
               w2, b2, g2, be2, m2, v2,
               w3, b3, g3, be3, m3, v3):
    """Quantize weights, fold BN, and lay out device arrays."""
    def quant(w):
        w = np.asarray(w, np.float32)
        s = np.median(np.abs(w).reshape(w.shape[0], -1), axis=1)
        s = np.maximum(s, np.float32(Q_EPS)).astype(np.float32)
        t = np.clip(np.round(w / s[:, None, None, None]), -1.0, 1.0).astype(np.float32)
        return t, s

    def fold(s, b, g, be, m, v):
        sc = np.asarray(g, np.float64) / np.sqrt(np.asarray(v, np.float64) + BN_EPS)
        a = (np.asarray(s, np.float64) * sc).astype(np.float32)
        c = (np.asarray(b, np.float64) * sc + np.asarray(be, np.float64)
             - np.asarray(m, np.float64) * sc).astype(np.float32)
        return a, c

    t1, s1 = quant(w1)
    t2, s2 = quant(w2)
    t3, s3 = quant(w3)
    a1, c1 = fold(s1, b1, g1, be1, m1, v1)
    a2, c2 = fold(s2, b2, g2, be2, m2, v2)
    a3, c3 = fold(s3, b3, g3, be3, m3, v3)

    # Fold c3 into the residual input: ship x' = x + c3 (broadcast per
    # channel; CIN == COUT so the same tensor serves conv1 and the residual).
    # conv1 then sees a per-channel constant shift, corrected exactly in c1:
    # conv1(x + c3) = conv1(x) + t1 @ c3  =>  c1 -= a1 * (t1 @ c3)
    t1c3 = t1[:, :, 0, 0].astype(np.float64) @ c3.astype(np.float64)
    c1 = (c1.astype(np.float64) - a1.astype(np.float64) * t1c3).astype(np.float32)

    bf = ml_dtypes.bfloat16
    fp8 = ml_dtypes.float8_e4m3

    def part_fold(m2d):
        # [K, M] -> [128, (K//128)*M]: row k*128+p lands at [p, k*M+m]
        kk, mm = m2d.shape
        return np.ascontiguousarray(
            m2d.reshape(kk // P, P, mm).transpose(1, 0, 2).reshape(P, -1))

    w1_dev = part_fold(t1[:, :, 0, 0].T).astype(fp8)
    # w2_dev[p, j, tap, e, m] = t2[j*128+m, e*128+p, dy, dx]
    w2_dev = np.ascontiguousarray(
        t2.reshape(W_T, P, W_T, P, 3, 3)           # j, m, e, p, dy, dx
        .transpose(3, 0, 4, 5, 2, 1)               # p, j, dy, dx, e, m
        .reshape(P, W_T * 9 * W_T * P)).astype(fp8)
    w3_dev = part_fold((t3[:, :, 0, 0] * a3[:, None]).T).astype(bf)

    cc = np.zeros((P, 8 + COUT_T), np.float32)
    cc[:, 0:2] = a1.reshape(W_T, P).T
    cc[:, 2:4] = c1.reshape(W_T, P).T
    cc[:, 4:6] = a2.reshape(W_T, P).T
    cc[:, 6:8] = c2.reshape(W_T, P).T
    cc[:, 8:] = c3.reshape(COUT_T, P).T

    const = {"w1": w1_dev, "w2": w2_dev, "w3": w3_dev,
             "cc": np.ascontiguousarray(cc)}

    x = np.asarray(x, np.float32) + c3[None, :, None, None]
    in_maps = []
    for c in range(N_CORES):
        # xd[p, k*BC*PIX + bp*NN + r*2W + cw*2 + i] = x[c*BC+bp*2+i, k*128+p, r, cw]
        xb = x[c * BC:(c + 1) * BC].reshape(BP, IPG, CIN_T, P, H, W)
        xc = np.ascontiguousarray(
            xb.transpose(3, 2, 0, 4, 5, 1).reshape(P, CIN_T * BC * PIX))
        # xq[p, kp, (bp r c i), e] = x'[ch=(4+2kp+e)*128+p] as e4m3
        xq = xb[:, :, 4:8].reshape(BP, IPG, 2, 2, P, H, W)
        xq = np.ascontiguousarray(
            xq.transpose(4, 2, 0, 5, 6, 1, 3).reshape(P, 2 * BC * PIX * 2))
        in_maps.append({"x": xc.astype(bf), "xq": xq.astype(fp8), **const})
    return in_maps


def _run(inputs, trace=False, act_func=None, **spmd_kwargs):
    nc = build(act_func)
    in_maps = _prep_host(**inputs)
    res = run_bass_kernel_spmd(nc, in_maps, list(range(N_CORES)),
                               trace=trace, **spmd_kwargs)
    outs = []
    for c in range(N_CORES):
        of = res.results[c]["out"].astype(np.float32)    # folded [P, COUT_T*BC*PIX]
        oc = of.reshape(P, COUT_T, BP, H, W, IPG)
        oc = oc.transpose(2, 5, 1, 0, 3, 4).reshape(BC, COUT, H, W)
        outs.append(oc)
    full = np.concatenate(outs, axis=0).astype(np.float32)
    return full, res


def kernel(**inputs):
    out, _ = _run(inputs)
    return out

